# revision 12
# baseline (speedup 1.0000x reference)
# Distributed causal multi-head attention kernel for one TRN2 chip (8 NeuronCores).
#
# Problem: x[2, 2048, 1024], 16 heads, head_dim 64, causal, MASK_VAL=-50000.
#   out = softmax(causal(q k^T / 8)) v @ Wo  with q = x Wq, (k|v) = x Wkv.
#
# Sharding (batch+head): core c handles batch c//4 and the 4 heads
# (c%4)*4 .. +4 (Wq/Wkv column-parallel, Wo row-parallel).  Each core writes
# a partial [2048, 1024] output (fp16); the host sums the 4 partials per
# batch.  No on-device collectives.
#
# Per-core layout strategy (all bf16 compute, f32 PSUM accumulate):
#   host feeds xT = x[b].T  -> projections need no on-device transpose:
#     qT[hd,n] = Wq_shard.T @ x.T : matmul(lhsT=Wq, rhs=xT)
#     kT[hd,n] likewise; v[n,hd] = matmul(lhsT=xT, rhs=Wv)
#   scoresT[j,i] = matmul(lhsT=kT block, rhs=qT block)   (K=hd=64)
#     - even/odd heads of a pair live at partitions 0:64 / 64:128; the two
#       K=64 matmuls are kept ADJACENT in the PE stream (explicit same-engine
#       ordering deps) so the hardware runs them concurrently in different
#       PE row groups.
#   softmax: no max subtraction needed (scores ~ N(0,1); exp(-50000) == 0.0
#     in f32 exactly, matching the reference's masked softmax).  exp on ACT
#     with scale=1/8 fused.  Row sums come for free: v is augmented with a
#     ones column (ones FIRST for odd heads), so the PV matmul accumulates
#     the softmax denominator in an extra partition row.
#   odd heads' PV output is placed at PSUM partitions 63:128 (base 63, den
#     row 63, v rows 64:128) so the normalized oT lands at partitions 64:128
#     without any partition-shift DMA.
#   causal: fully-masked j-blocks skipped; diagonal blocks compute only the
#     live column range and apply a 128x128 triangular 0/1 mask (host input).
#   normalize: denominator row -> [64,8] via SBUF DMA, DVE reciprocal in
#     parallel lanes, row broadcast via GPSIMD partition_broadcast, bf16 DVE
#     multiply (2x mode).
#   out = matmul(lhsT=oT, rhs=Wo_shard), streamed out per 128-row chunk as
#     fp16 partials.
#
# Scheduling: the whole PE instruction stream is pinned into emission order
# with sync=False ordering deps; projection/Wo work is chopped into ~0.5-1us
# "filler" units paced into the attention loop (which is ACT-exp-bound at
# ~950ns/block) so the PE never idles and HAM stays at full clock.

import os

import numpy as np
import ml_dtypes

import concourse.bass as bass
import concourse.mybir as mybir
import concourse.tile as tile
from concourse.bass_utils import run_bass_kernel_spmd


def _install_axon_ntff_shim():
    """This container's `antenv` lacks `axon_hooks`, which bass_utils imports
    when tracing under axon.  Provide the module and install the ctypes NTFF
    hook against libaxon_pjrt.so so BASS_TRACE=1 profiling works."""
    import sys
    import types
    import contextlib
    import ctypes
    try:
        import antenv.axon_hooks  # noqa: F401
        return
    except ImportError:
        pass
    try:
        import antenv
    except ImportError:
        return
    mod = types.ModuleType("antenv.axon_hooks")
    state = {"hook": None}
    mod.set_axon_ntff_profile_hook = lambda h: state.__setitem__("hook", h)
    mod.get_axon_ntff_profile_hook = lambda: state["hook"]
    sys.modules["antenv.axon_hooks"] = mod
    antenv.axon_hooks = mod
    so_path = "/opt/axon/libaxon_pjrt.so"
    try:
        lib = ctypes.CDLL(so_path)
        if not hasattr(lib, "axon_start_nrt_profile"):
            return
        lib.axon_start_nrt_profile.argtypes = [
            ctypes.POINTER(ctypes.c_int64), ctypes.c_size_t]
        lib.axon_start_nrt_profile.restype = ctypes.c_int64
        lib.axon_stop_nrt_profile.argtypes = [ctypes.c_char_p]
        lib.axon_stop_nrt_profile.restype = ctypes.c_int64

        @contextlib.contextmanager
        def _hook(output_dir, device_ids):
            import jax
            jax.devices()
            if device_ids:
                ids = (ctypes.c_int64 * len(device_ids))(*device_ids)
                rc = lib.axon_start_nrt_profile(ids, len(device_ids))
            else:
                rc = lib.axon_start_nrt_profile(None, 0)
            if rc != 0:
                raise RuntimeError(f"axon_start_nrt_profile rc={rc}")
            try:
                yield
            finally:
                n = lib.axon_stop_nrt_profile(str(output_dir).encode())
                print(f"ntff profile: {n} file(s) -> {output_dir}")

        mod.set_axon_ntff_profile_hook(_hook)
    except Exception:
        pass


_install_axon_ntff_shim()

BF16 = ml_dtypes.bfloat16
P = 128
N = 2048          # sequence length
D = 1024          # model dim
HD = 64           # head dim
HL = 4            # local heads per core
DQ = HL * HD      # 256 local projection width
KC = D // P       # 8 contraction chunks
NPAIR = HL // 2   # head pairs (even@part 0:64, odd@part 64:128)
IC = 512          # i-chunk (query) width
NIC = N // IC     # 4
NJB = N // P      # 16 j-blocks
F32 = mybir.dt.float32
F16 = mybir.dt.float16
BF = mybir.dt.bfloat16

LAST_RESULT = {}


def build_nc():
    nc = bass.Bass()
    xtq = nc.declare_dram_parameter("xtq", [NIC, P, KC * IC], BF, isOutput=False)
    wqkv = nc.declare_dram_parameter("wqkv", [P, KC, 3 * DQ], BF, isOutput=False)
    wo = nc.declare_dram_parameter("wo", [P, 2, D], BF, isOutput=False)
    mask = nc.declare_dram_parameter("mask", [P, 2, P], BF, isOutput=False)
    out = nc.declare_dram_parameter("out", [N, D], F16, isOutput=True)

    Exp = mybir.ActivationFunctionType.Exp

    # pin every PE matmul into emission order so paired K=64 score matmuls
    # stay adjacent (-> concurrent row-group execution) and filler pacing is
    # deterministic.
    pe_prev = [None]

    with tile.TileContext(nc) as tc:
        def mm(*args, **kw):
            inst = nc.tensor.matmul(*args, **kw).ins
            if pe_prev[0] is not None:
                # add_dep_helper(a, b): a depends on (waits for) b
                tile.add_dep_helper(inst, pe_prev[0], sync=False,
                                    reason="pe stream order")
            pe_prev[0] = inst
            return inst

        with (
            tc.tile_pool(name="const", bufs=1) as constp,
            tc.tile_pool(name="expp", bufs=4) as expp,
            tc.tile_pool(name="normp", bufs=2) as normp,
            tc.tile_pool(name="outp", bufs=3) as outp,
            tc.tile_pool(name="psS", bufs=2, space="PSUM") as psS,
            tc.tile_pool(name="psO", bufs=1, space="PSUM") as psO,
            tc.tile_pool(name="psM", bufs=2, space="PSUM") as psM,
        ):
            # ---------------- resident SBUF tensors + input DMA ----------------
            xT_sb = constp.tile([P, KC, N], BF, tag="xT")
            wqkv_sb = constp.tile([P, KC, 3 * DQ], BF, tag="wqkv")
            wo_sb = constp.tile([P, 2, D], BF, tag="wo")
            mask_sb = constp.tile([P, 2, P], BF, tag="mask")
            qT_sb = constp.tile([P, NPAIR, N], BF, tag="qT")
            kT_sb = constp.tile([P, NPAIR, N], BF, tag="kT")
            # v, head-major, with a ones column appended per head (PV row 64
            # then accumulates the softmax denominator for free).
            v_sb = constp.tile([P, NJB, HL, HD + 1], BF, tag="v")
            oT_sb = constp.tile([P, NPAIR, N], BF, tag="oT")
            # bf16 ones row at partition 64 for the reciprocal-broadcast
            # outer product (lhsT/rhs of a K=1 matmul must share a base
            # partition; the recip row lives at partition 64).
            ones_sb = constp.tile([P, HD], BF, tag="ones")
            nc.vector.memset(ones_sb[HD:HD + 1, :], 1.0)

            # weights ride the SP(sync) HWDGE ring; xT quarters ride the
            # ACT(scalar) ring.  Host-side staging makes every transfer one
            # big contiguous descriptor set.
            nc.sync.dma_start(wqkv_sb[:, :, 0:DQ], wqkv[:, :, 0:DQ])
            nc.sync.dma_start(wqkv_sb[:, :, DQ:2 * DQ], wqkv[:, :, DQ:2 * DQ])
            nc.sync.dma_start(wqkv_sb[:, :, 2 * DQ:3 * DQ],
                              wqkv[:, :, 2 * DQ:3 * DQ])
            nc.sync.dma_start(mask_sb[:], mask[:, :, :])
            nc.sync.dma_start(wo_sb[:], wo[:, :, :])
            for q in range(NIC):
                nc.scalar.dma_start(xT_sb[:, :, q * IC:(q + 1) * IC], xtq[q])

            nc.vector.memset(v_sb[:, :, :, HD], 1.0)

            # ---------------- projections (unit-granular for pacing) --------
            def proj_qk_units(pair, i4, which):
                off = which * DQ
                dst = qT_sb if which == 0 else kT_sb
                st = {}

                def unit_a():
                    st["ps"] = psM.tile([P, IC], F32, tag="mm", name="mm")
                    for kc in range(4):
                        mm(st["ps"][:],
                           wqkv_sb[:, kc, off + pair * P:off + (pair + 1) * P],
                           xT_sb[:, kc, i4 * IC:(i4 + 1) * IC],
                           start=(kc == 0), stop=False)

                def unit_b():
                    for kc in range(4, KC):
                        mm(st["ps"][:],
                           wqkv_sb[:, kc, off + pair * P:off + (pair + 1) * P],
                           xT_sb[:, kc, i4 * IC:(i4 + 1) * IC],
                           start=False, stop=(kc == KC - 1))
                    nc.vector.tensor_copy(
                        dst[:, pair, i4 * IC:(i4 + 1) * IC], st["ps"][:])

                return [unit_a, unit_b]

            def proj_v_unit(jc):
                def u():
                    ps = psM.tile([P, IC], F32, tag="mm", name="mm")
                    for kc in range(KC):
                        mm(ps[:, :DQ],
                           xT_sb[:, kc, jc * P:(jc + 1) * P],
                           wqkv_sb[:, kc, 2 * DQ:3 * DQ],
                           start=(kc == 0), stop=(kc == KC - 1))
                    nc.vector.tensor_copy(
                        v_sb[:, jc, :, 0:HD],
                        ps[:, :DQ].rearrange("p (h e) -> p h e", e=HD))
                return [u]

            def wo_units(mc):
                st = {}

                def ua():
                    st["osb"] = outp.tile([P, D], F16, tag="osb", name="osb")
                    ps = psM.tile([P, IC], F32, tag="mm", name="mm")
                    for kc2 in range(2):
                        mm(ps[:],
                           oT_sb[:, kc2, mc * P:(mc + 1) * P],
                           wo_sb[:, kc2, 0:IC],
                           start=(kc2 == 0), stop=(kc2 == 1))
                    nc.vector.tensor_copy(st["osb"][:, 0:IC], ps[:])

                def ub():
                    ps = psM.tile([P, IC], F32, tag="mm", name="mm")
                    for kc2 in range(2):
                        mm(ps[:],
                           oT_sb[:, kc2, mc * P:(mc + 1) * P],
                           wo_sb[:, kc2, IC:D],
                           start=(kc2 == 0), stop=(kc2 == 1))
                    nc.vector.tensor_copy(st["osb"][:, IC:D], ps[:])
                    nc.sync.dma_start(out[mc * P:(mc + 1) * P, :], st["osb"][:])

                return [ua, ub]

            # prologue: pair-0 q/k projections for i-chunk 0 and v for the
            # first 4 j-blocks run as soon as their DMA slices land.
            for u in proj_qk_units(0, 0, 0):
                u()
            for u in proj_qk_units(0, 0, 1):
                u()
            for jc in range(4):
                for u in proj_v_unit(jc):
                    u()

            # ---------------- attention ----------------
            def normalize(pair, i4, po):
                for h01 in range(2):
                    posb = normp.tile([HD + 1, IC], F32, tag=f"posb{h01}",
                                      name=f"posb{h01}")
                    nc.vector.tensor_copy(posb[:], po[h01][0:HD + 1, :])
                    # reciprocal of the 512 sums in parallel lanes: reshape
                    # to [64, 8] via SBUF->SBUF DMA (equal element counts),
                    # DVE reciprocal, reshape back to a row at partition 64.
                    with nc.allow_low_precision(
                            "softmax denominators are well-conditioned"):
                        sT = normp.tile([HD, 8], F32, tag="sT")
                        nc.sync.dma_start(sT[:], posb[HD:HD + 1, :])
                        rT = normp.tile([HD, 8], BF, tag="rT")
                        nc.vector.reciprocal(rT[:], sT[:])
                        rc = normp.tile([P, IC], BF, tag="rc")
                        nc.sync.dma_start(rc[HD:HD + 1, :], rT[:])
                    # broadcast recip row across 64 partitions via a K=1
                    # outer product on PE (shares the psM "mm" slots)
                    bc = psM.tile([HD, IC], F32, tag="mm", name="bc")
                    mm(bc[:], ones_sb[HD:HD + 1, 0:HD], rc[HD:HD + 1, :],
                       start=True, stop=True)
                    if h01 == 0:
                        nc.vector.tensor_mul(
                            oT_sb[0:HD, pair, i4 * IC:(i4 + 1) * IC],
                            posb[0:HD, :], bc[:])
                    else:
                        # odd head must land at partitions 64:128 for the Wo
                        # matmul; DVE lanes are partition-locked, so an
                        # SBUF->SBUF DMA does the partition shift.
                        ot = normp.tile([HD, IC], BF, tag="otmp")
                        nc.vector.tensor_mul(ot[:], posb[0:HD, :], bc[:])
                        nc.sync.dma_start(
                            oT_sb[HD:P, pair, i4 * IC:(i4 + 1) * IC], ot[:])

            for i4 in range(NIC):
                fillers = []
                if i4 == 0:
                    for which in range(2):
                        fillers += proj_qk_units(1, 0, which)
                if i4 + 1 < NIC:
                    for pair in range(NPAIR):
                        for which in range(2):
                            fillers += proj_qk_units(pair, i4 + 1, which)
                    for jc in range(4 * (i4 + 1), 4 * (i4 + 1) + 4):
                        fillers += proj_v_unit(jc)
                if i4 >= 1:
                    for mc in range(4 * (i4 - 1), 4 * (i4 - 1) + 4):
                        fillers += wo_units(mc)
                fi = 0
                it = 0
                nblocks = 4 * i4 + 4
                n_slots = NPAIR * (nblocks - 2)

                for pair in range(NPAIR):
                    po = [
                        psO.tile([P, IC], F32, tag=f"po{h01}", name=f"po{h01}")
                        for h01 in range(2)
                    ]
                    eTs = {}

                    def scores_exp(jb, pair=pair, i4=i4, eTs=eTs):
                        r = jb - 4 * i4  # >=0 -> diagonal block
                        lo = max(0, r * P)
                        pss = psS.tile([P, 2, IC], F32, tag="pss", name="pss")
                        for h01 in range(2):
                            pb = h01 * HD
                            mm(
                                pss[:, h01, lo:IC],
                                kT_sb[pb:pb + HD, pair, jb * P:(jb + 1) * P],
                                qT_sb[pb:pb + HD, pair,
                                      i4 * IC + lo:(i4 + 1) * IC],
                                start=True, stop=True,
                            )
                        eT = expp.tile([P, 2, IC], BF, tag="eT", name="eT")
                        nc.scalar.activation(
                            eT[:, :, lo:IC], pss[:, :, lo:IC], Exp, scale=0.125
                        )
                        if r >= 0:
                            nc.vector.tensor_mul(
                                eT[:, :, lo:lo + P], eT[:, :, lo:lo + P],
                                mask_sb[:]
                            )
                        eTs[jb] = (eT, lo)

                    def pv(jb, pair=pair, po=po, nblocks=nblocks, eTs=eTs):
                        eT, lo = eTs.pop(jb)
                        for h01 in range(2):
                            mm(
                                po[h01][0:HD + 1, lo:IC],
                                v_sb[:, jb, 2 * pair + h01, :],
                                eT[:, h01, lo:IC],
                                start=(jb == 0), stop=(jb == nblocks - 1),
                            )

                    scores_exp(0)
                    scores_exp(1)
                    for jb in range(2, nblocks):
                        scores_exp(jb)
                        pv(jb - 2)
                        it += 1
                        while (fi < len(fillers)
                               and fi * n_slots <= it * len(fillers)):
                            fillers[fi]()
                            fi += 1
                    pv(nblocks - 2)
                    pv(nblocks - 1)
                    normalize(pair, i4, po)
                # drain any leftover fillers for this i-chunk
                while fi < len(fillers):
                    fillers[fi]()
                    fi += 1
            # last i-chunk's Wo
            for mc in range(4 * (NIC - 1), 4 * NIC):
                for u in wo_units(mc):
                    u()
    return nc


_LEGALIZE_TYPES = None


def _legalize_pe_waits(nc, max_waits=1):
    """walrus' TPB instruction encodings fit very few semaphore waits
    (Matmult: 1; TensorTensor etc. similarly limited) but Tile sometimes
    emits more.  Move the excess onto an InstNoOp inserted just before the
    instruction in the same engine stream — waiting earlier on the same
    engine is always safe."""
    global _LEGALIZE_TYPES
    if _LEGALIZE_TYPES is None:
        _LEGALIZE_TYPES = (
            mybir.InstMatmult, mybir.InstLdweights, mybir.InstTensorTensor,
            mybir.InstTensorCopy, mybir.InstActivation, mybir.InstReciprocal,
            mybir.InstMemset, mybir.InstTensorReduce, mybir.InstIota,
            mybir.InstTensorScalarPtr, mybir.InstISA, mybir.InstDMACopy,
            mybir.InstTensorTensorReduce, mybir.InstDrain,
            mybir.InstDmaTransposeAnt,
        )
    n_fixed = 0
    for fn in nc.m.functions:
        for blk in fn.blocks:
            insts = list(blk.instructions)
            out = []
            for inst in insts:
                si = getattr(inst, "sync_info", None)
                if (
                    isinstance(inst, _LEGALIZE_TYPES)
                    and si is not None
                    and si.on_wait
                    and len(si.on_wait) > max_waits
                ):
                    extra = list(si.on_wait[:-max_waits])
                    keep = list(si.on_wait[-max_waits:])
                    for w in extra:
                        out.append(mybir.InstEventSemaphore(
                            name=nc.get_next_instruction_name(),
                            engine=inst.engine,
                            ins=[],
                            outs=[],
                            sync_info=mybir.SyncInfo(on_wait=[w], on_update=[]),
                            bass_nofuse=True,
                        ))
                    inst.sync_info = mybir.SyncInfo(
                        on_wait=keep, on_update=list(si.on_update)
                    )
                    n_fixed += 1
                out.append(inst)
            blk.instructions = out
    return n_fixed


_NC_CACHE = {}


def _get_nc():
    if "nc" not in _NC_CACHE:
        nc = build_nc()
        _legalize_pe_waits(nc)
        _NC_CACHE["nc"] = nc
    return _NC_CACHE["nc"]


def _make_mask():
    tri = np.triu(np.ones((P, P), np.float32))  # keep j<=c
    return np.ascontiguousarray(
        np.broadcast_to(tri[:, None, :], (P, 2, P))
    ).astype(BF16)


def kernel(x, Wq, Wkv, Wo, **kw):
    x = np.asarray(x, np.float32)
    Wq = np.asarray(Wq, np.float32)
    Wkv = np.asarray(Wkv, np.float32)
    Wo = np.asarray(Wo, np.float32)
    mask = _make_mask()

    in_maps = []
    for c in range(8):
        b = c // 4
        hs = (c % 4) * DQ
        xb = x[b]  # [2048, 1024]
        # xtq[q, p, kc, i] = x[b, q*512+i, kc*128+p]
        xtq = xb.reshape(NIC, IC, KC, P).transpose(0, 3, 2, 1)
        xtq = np.ascontiguousarray(xtq).reshape(NIC, P, KC * IC)
        # wqkv[p, kc, :] = [Wq | Wk | Wv][kc*128+p, shard cols]
        wq_s = Wq[:, hs:hs + DQ].reshape(KC, P, DQ).transpose(1, 0, 2)
        wk_s = Wkv[:, hs:hs + DQ].reshape(KC, P, DQ).transpose(1, 0, 2)
        wv_s = Wkv[:, D + hs:D + hs + DQ].reshape(KC, P, DQ).transpose(1, 0, 2)
        wqkv_h = np.ascontiguousarray(
            np.concatenate([wq_s, wk_s, wv_s], axis=2))
        # wo[p, c2, :] = Wo[hs + c2*128 + p, :]
        wo_s = np.ascontiguousarray(
            Wo[hs:hs + DQ, :].reshape(2, P, D).transpose(1, 0, 2))
        in_maps.append({
            "xtq": xtq.astype(BF16),
            "wqkv": wqkv_h.astype(BF16),
            "wo": wo_s.astype(BF16),
            "mask": mask,
        })

    res = run_bass_kernel_spmd(_get_nc(), in_maps, core_ids=list(range(8)))
    LAST_RESULT["exec_time_ns"] = res.exec_time_ns
    LAST_RESULT["trace"] = res.instructions_and_trace
    LAST_RESULT["profile_json"] = res.profile_json
    parts = [np.asarray(r["out"], np.float32) for r in res.results]
    out = np.stack(
        [parts[0] + parts[1] + parts[2] + parts[3],
         parts[4] + parts[5] + parts[6] + parts[7]], axis=0
    )
    return out


# revision 15
# speedup vs baseline: 1.0994x; 1.0994x over previous
# Distributed causal multi-head attention kernel for one TRN2 chip (8 NeuronCores).
#
# Problem: x[2, 2048, 1024], 16 heads, head_dim 64, causal, MASK_VAL=-50000.
#   out = softmax(causal(q k^T / 8)) v @ Wo  with q = x Wq, (k|v) = x Wkv.
#
# Sharding (batch+head): core c handles batch c//4 and the 4 heads
# (c%4)*4 .. +4 (Wq/Wkv column-parallel, Wo row-parallel).  Each core writes
# a partial [2048, 1024] output (fp16); the host sums the 4 partials per
# batch.  No on-device collectives.
#
# Per-core layout strategy (all bf16 compute, f32 PSUM accumulate):
#   host feeds xT = x[b].T  -> projections need no on-device transpose:
#     qT[hd,n] = Wq_shard.T @ x.T : matmul(lhsT=Wq, rhs=xT)
#     kT[hd,n] likewise; v[n,hd] = matmul(lhsT=xT, rhs=Wv)
#   scoresT[j,i] = matmul(lhsT=kT block, rhs=qT block)   (K=hd=64)
#     - even/odd heads of a pair live at partitions 0:64 / 64:128; the two
#       K=64 matmuls are kept ADJACENT in the PE stream (explicit same-engine
#       ordering deps) so the hardware runs them concurrently in different
#       PE row groups.
#   softmax: no max subtraction needed (scores ~ N(0,1); exp(-50000) == 0.0
#     in f32 exactly, matching the reference's masked softmax).  exp on ACT
#     with scale=1/8 fused.  Row sums come for free: v is augmented with a
#     ones column (ones FIRST for odd heads), so the PV matmul accumulates
#     the softmax denominator in an extra partition row.
#   odd heads' PV output is placed at PSUM partitions 63:128 (base 63, den
#     row 63, v rows 64:128) so the normalized oT lands at partitions 64:128
#     without any partition-shift DMA.
#   causal: fully-masked j-blocks skipped; diagonal blocks compute only the
#     live column range and apply a 128x128 triangular 0/1 mask (host input).
#   normalize: denominator row -> [64,8] via SBUF DMA, DVE reciprocal in
#     parallel lanes, row broadcast via GPSIMD partition_broadcast, bf16 DVE
#     multiply (2x mode).
#   out = matmul(lhsT=oT, rhs=Wo_shard), streamed out per 128-row chunk as
#     fp16 partials.
#
# Scheduling: the whole PE instruction stream is pinned into emission order
# with sync=False ordering deps; projection/Wo work is chopped into ~0.5-1us
# "filler" units paced into the attention loop (which is ACT-exp-bound at
# ~950ns/block) so the PE never idles and HAM stays at full clock.

import os

import numpy as np
import ml_dtypes

import concourse.bass as bass
import concourse.mybir as mybir
import concourse.tile as tile
from concourse.bass_utils import run_bass_kernel_spmd


def _install_axon_ntff_shim():
    """This container's `antenv` lacks `axon_hooks`, which bass_utils imports
    when tracing under axon.  Provide the module and install the ctypes NTFF
    hook against libaxon_pjrt.so so BASS_TRACE=1 profiling works."""
    import sys
    import types
    import contextlib
    import ctypes
    try:
        import antenv.axon_hooks  # noqa: F401
        return
    except ImportError:
        pass
    try:
        import antenv
    except ImportError:
        return
    mod = types.ModuleType("antenv.axon_hooks")
    state = {"hook": None}
    mod.set_axon_ntff_profile_hook = lambda h: state.__setitem__("hook", h)
    mod.get_axon_ntff_profile_hook = lambda: state["hook"]
    sys.modules["antenv.axon_hooks"] = mod
    antenv.axon_hooks = mod
    so_path = "/opt/axon/libaxon_pjrt.so"
    try:
        lib = ctypes.CDLL(so_path)
        if not hasattr(lib, "axon_start_nrt_profile"):
            return
        lib.axon_start_nrt_profile.argtypes = [
            ctypes.POINTER(ctypes.c_int64), ctypes.c_size_t]
        lib.axon_start_nrt_profile.restype = ctypes.c_int64
        lib.axon_stop_nrt_profile.argtypes = [ctypes.c_char_p]
        lib.axon_stop_nrt_profile.restype = ctypes.c_int64

        @contextlib.contextmanager
        def _hook(output_dir, device_ids):
            import jax
            jax.devices()
            if device_ids:
                ids = (ctypes.c_int64 * len(device_ids))(*device_ids)
                rc = lib.axon_start_nrt_profile(ids, len(device_ids))
            else:
                rc = lib.axon_start_nrt_profile(None, 0)
            if rc != 0:
                raise RuntimeError(f"axon_start_nrt_profile rc={rc}")
            try:
                yield
            finally:
                n = lib.axon_stop_nrt_profile(str(output_dir).encode())
                print(f"ntff profile: {n} file(s) -> {output_dir}")

        mod.set_axon_ntff_profile_hook(_hook)
    except Exception:
        pass


_install_axon_ntff_shim()

BF16 = ml_dtypes.bfloat16
P = 128
N = 2048          # sequence length
D = 1024          # model dim
HD = 64           # head dim
HL = 4            # local heads per core
DQ = HL * HD      # 256 local projection width
KC = D // P       # 8 contraction chunks
NPAIR = HL // 2   # head pairs (even@part 0:64, odd@part 64:128)
IC = 512          # i-chunk (query) width
NIC = N // IC     # 4
NJB = N // P      # 16 j-blocks
F32 = mybir.dt.float32
F16 = mybir.dt.float16
BF = mybir.dt.bfloat16

LAST_RESULT = {}


def build_nc():
    nc = bass.Bass()
    xtq = nc.declare_dram_parameter("xtq", [NIC, P, KC * IC], BF, isOutput=False)
    wqkv = nc.declare_dram_parameter("wqkv", [P, KC, 3 * DQ], BF, isOutput=False)
    wo = nc.declare_dram_parameter("wo", [P, 2, D], BF, isOutput=False)
    mask = nc.declare_dram_parameter("mask", [P, 2, P], BF, isOutput=False)
    out = nc.declare_dram_parameter("out", [N, D], F16, isOutput=True)

    Exp = mybir.ActivationFunctionType.Exp

    # pin every PE matmul into emission order so paired K=64 score matmuls
    # stay adjacent (-> concurrent row-group execution) and filler pacing is
    # deterministic.
    pe_prev = [None]

    with tile.TileContext(nc) as tc:
        def mm(*args, **kw):
            inst = nc.tensor.matmul(*args, **kw).ins
            if pe_prev[0] is not None:
                # add_dep_helper(a, b): a depends on (waits for) b
                tile.add_dep_helper(inst, pe_prev[0], sync=False,
                                    reason="pe stream order")
            pe_prev[0] = inst
            return inst

        with (
            tc.tile_pool(name="const", bufs=1) as constp,
            tc.tile_pool(name="expp", bufs=6) as expp,
            tc.tile_pool(name="normp", bufs=2) as normp,
            tc.tile_pool(name="outp", bufs=3) as outp,
            tc.tile_pool(name="psS", bufs=2, space="PSUM") as psS,
            tc.tile_pool(name="psO", bufs=1, space="PSUM") as psO,
            tc.tile_pool(name="psM", bufs=2, space="PSUM") as psM,
        ):
            # ---------------- resident SBUF tensors + input DMA ----------------
            xT_sb = constp.tile([P, KC, N], BF, tag="xT")
            wqkv_sb = constp.tile([P, KC, 3 * DQ], BF, tag="wqkv")
            wo_sb = constp.tile([P, 2, D], BF, tag="wo")
            mask_sb = constp.tile([P, 2, P], BF, tag="mask")
            qT_sb = constp.tile([P, NPAIR, N], BF, tag="qT")
            kT_sb = constp.tile([P, NPAIR, N], BF, tag="kT")
            # v, head-major, with a ones column appended per head (PV row 64
            # then accumulates the softmax denominator for free).
            v_sb = constp.tile([P, NJB, HL, HD + 1], BF, tag="v")
            oT_sb = constp.tile([P, NPAIR, N], BF, tag="oT")
            # bf16 ones row at partition 64 for the reciprocal-broadcast
            # outer product (lhsT/rhs of a K=1 matmul must share a base
            # partition; the recip row lives at partition 64).
            ones_sb = constp.tile([P, HD], BF, tag="ones")
            nc.vector.memset(ones_sb[HD:HD + 1, :], 1.0)

            # ALL input DMAs ride one HWDGE ring (SP/sync): a single ring
            # feeds all 16 SDMA engines at full HBM rate and drains FIFO, so
            # issue order IS priority order — first-needed data arrives
            # first instead of competing with later transfers.
            nc.sync.dma_start(wqkv_sb[:, :, 0:DQ], wqkv[:, :, 0:DQ])
            nc.sync.dma_start(xT_sb[:, :, 0:IC], xtq[0])
            nc.sync.dma_start(wqkv_sb[:, :, DQ:2 * DQ], wqkv[:, :, DQ:2 * DQ])
            nc.sync.dma_start(wqkv_sb[:, :, 2 * DQ:3 * DQ],
                              wqkv[:, :, 2 * DQ:3 * DQ])
            nc.sync.dma_start(xT_sb[:, :, IC:2 * IC], xtq[1])
            nc.sync.dma_start(mask_sb[:], mask[:, :, :])
            nc.sync.dma_start(wo_sb[:], wo[:, :, :])
            for q in range(2, NIC):
                nc.sync.dma_start(xT_sb[:, :, q * IC:(q + 1) * IC], xtq[q])

            nc.vector.memset(v_sb[:, :, :, HD], 1.0)

            # ---------------- projections (unit-granular for pacing) --------
            def proj_qk_units(pair, i4, which):
                off = which * DQ
                dst = qT_sb if which == 0 else kT_sb
                st = {}

                def unit_a():
                    st["ps"] = psM.tile([P, IC], F32, tag="mm", name="mm")
                    for kc in range(4):
                        mm(st["ps"][:],
                           wqkv_sb[:, kc, off + pair * P:off + (pair + 1) * P],
                           xT_sb[:, kc, i4 * IC:(i4 + 1) * IC],
                           start=(kc == 0), stop=False)

                def unit_b():
                    for kc in range(4, KC):
                        mm(st["ps"][:],
                           wqkv_sb[:, kc, off + pair * P:off + (pair + 1) * P],
                           xT_sb[:, kc, i4 * IC:(i4 + 1) * IC],
                           start=False, stop=(kc == KC - 1))
                    nc.vector.tensor_copy(
                        dst[:, pair, i4 * IC:(i4 + 1) * IC], st["ps"][:])

                return [unit_a, unit_b]

            def proj_v_unit(jc):
                def u():
                    ps = psM.tile([P, IC], F32, tag="mm", name="mm")
                    for kc in range(KC):
                        mm(ps[:, :DQ],
                           xT_sb[:, kc, jc * P:(jc + 1) * P],
                           wqkv_sb[:, kc, 2 * DQ:3 * DQ],
                           start=(kc == 0), stop=(kc == KC - 1))
                    nc.vector.tensor_copy(
                        v_sb[:, jc, :, 0:HD],
                        ps[:, :DQ].rearrange("p (h e) -> p h e", e=HD))
                return [u]

            def wo_units(mc):
                st = {}

                def ua():
                    st["osb"] = outp.tile([P, D], F16, tag="osb", name="osb")
                    ps = psM.tile([P, IC], F32, tag="mm", name="mm")
                    for kc2 in range(2):
                        mm(ps[:],
                           oT_sb[:, kc2, mc * P:(mc + 1) * P],
                           wo_sb[:, kc2, 0:IC],
                           start=(kc2 == 0), stop=(kc2 == 1))
                    nc.vector.tensor_copy(st["osb"][:, 0:IC], ps[:])

                def ub():
                    ps = psM.tile([P, IC], F32, tag="mm", name="mm")
                    for kc2 in range(2):
                        mm(ps[:],
                           oT_sb[:, kc2, mc * P:(mc + 1) * P],
                           wo_sb[:, kc2, IC:D],
                           start=(kc2 == 0), stop=(kc2 == 1))
                    nc.vector.tensor_copy(st["osb"][:, IC:D], ps[:])
                    nc.sync.dma_start(out[mc * P:(mc + 1) * P, :], st["osb"][:])

                return [ua, ub]

            # prologue: pair-0 q/k projections for i-chunk 0 and v for the
            # first 4 j-blocks run as soon as their DMA slices land.
            for u in proj_qk_units(0, 0, 0):
                u()
            for u in proj_qk_units(0, 0, 1):
                u()
            for jc in range(4):
                for u in proj_v_unit(jc):
                    u()

            # ---------------- attention ----------------
            def normalize(pair, i4, po):
                for h01 in range(2):
                    posb = normp.tile([HD + 1, IC], F32, tag=f"posb{h01}",
                                      name=f"posb{h01}")
                    nc.vector.tensor_copy(posb[:], po[h01][0:HD + 1, :])
                    # reciprocal of the 512 sums in parallel lanes: reshape
                    # to [64, 8] via SBUF->SBUF DMA (equal element counts),
                    # DVE reciprocal, reshape back to a row at partition 64.
                    with nc.allow_low_precision(
                            "softmax denominators are well-conditioned"):
                        sT = normp.tile([HD, 8], F32, tag="sT")
                        nc.sync.dma_start(sT[:], posb[HD:HD + 1, :])
                        rT = normp.tile([HD, 8], BF, tag="rT")
                        nc.vector.reciprocal(rT[:], sT[:])
                        rc = normp.tile([P, IC], BF, tag="rc")
                        nc.sync.dma_start(rc[HD:HD + 1, :], rT[:])
                    # broadcast recip row across 64 partitions via a K=1
                    # outer product on PE (shares the psM "mm" slots).
                    # NOT chained into the PE stream order: the ~5us recip
                    # chain latency must not stall later scores/PV matmuls —
                    # the scheduler floats this matmul wherever it fits.
                    bc = psM.tile([HD, IC], F32, tag="mm", name="bc")
                    nc.tensor.matmul(
                        bc[:], ones_sb[HD:HD + 1, 0:HD], rc[HD:HD + 1, :],
                        start=True, stop=True)
                    if h01 == 0:
                        nc.vector.tensor_mul(
                            oT_sb[0:HD, pair, i4 * IC:(i4 + 1) * IC],
                            posb[0:HD, :], bc[:])
                    else:
                        # odd head must land at partitions 64:128 for the Wo
                        # matmul; DVE lanes are partition-locked, so an
                        # SBUF->SBUF DMA does the partition shift.
                        ot = normp.tile([HD, IC], BF, tag="otmp")
                        nc.vector.tensor_mul(ot[:], posb[0:HD, :], bc[:])
                        nc.sync.dma_start(
                            oT_sb[HD:P, pair, i4 * IC:(i4 + 1) * IC], ot[:])

            for i4 in range(NIC):
                fillers = []
                if i4 == 0:
                    for which in range(2):
                        fillers += proj_qk_units(1, 0, which)
                if i4 + 1 < NIC:
                    for pair in range(NPAIR):
                        for which in range(2):
                            fillers += proj_qk_units(pair, i4 + 1, which)
                    for jc in range(4 * (i4 + 1), 4 * (i4 + 1) + 4):
                        fillers += proj_v_unit(jc)
                if i4 >= 1:
                    for mc in range(4 * (i4 - 1), 4 * (i4 - 1) + 4):
                        fillers += wo_units(mc)
                fi = 0
                it = 0
                nblocks = 4 * i4 + 4
                n_slots = NPAIR * (nblocks - 2)

                for pair in range(NPAIR):
                    po = [
                        psO.tile([P, IC], F32, tag=f"po{h01}", name=f"po{h01}")
                        for h01 in range(2)
                    ]
                    eTs = {}

                    def scores_exp(jb, pair=pair, i4=i4, eTs=eTs):
                        r = jb - 4 * i4  # >=0 -> diagonal block
                        lo = max(0, r * P)
                        pss = psS.tile([P, 2, IC], F32, tag="pss", name="pss")
                        for h01 in range(2):
                            pb = h01 * HD
                            mm(
                                pss[:, h01, lo:IC],
                                kT_sb[pb:pb + HD, pair, jb * P:(jb + 1) * P],
                                qT_sb[pb:pb + HD, pair,
                                      i4 * IC + lo:(i4 + 1) * IC],
                                start=True, stop=True,
                            )
                        eT = expp.tile([P, 2, IC], BF, tag="eT", name="eT")
                        nc.scalar.activation(
                            eT[:, :, lo:IC], pss[:, :, lo:IC], Exp, scale=0.125
                        )
                        if r >= 0:
                            nc.vector.tensor_mul(
                                eT[:, :, lo:lo + P], eT[:, :, lo:lo + P],
                                mask_sb[:]
                            )
                        eTs[jb] = (eT, lo)

                    def pv(jb, pair=pair, po=po, nblocks=nblocks, eTs=eTs):
                        eT, lo = eTs.pop(jb)
                        for h01 in range(2):
                            mm(
                                po[h01][0:HD + 1, lo:IC],
                                v_sb[:, jb, 2 * pair + h01, :],
                                eT[:, h01, lo:IC],
                                start=(jb == 0), stop=(jb == nblocks - 1),
                            )

                    scores_exp(0)
                    scores_exp(1)
                    for jb in range(2, nblocks):
                        scores_exp(jb)
                        pv(jb - 2)
                        it += 1
                        while (fi < len(fillers)
                               and fi * n_slots <= it * len(fillers)):
                            fillers[fi]()
                            fi += 1
                    pv(nblocks - 2)
                    pv(nblocks - 1)
                    normalize(pair, i4, po)
                # drain any leftover fillers for this i-chunk
                while fi < len(fillers):
                    fillers[fi]()
                    fi += 1
            # last i-chunk's Wo
            for mc in range(4 * (NIC - 1), 4 * NIC):
                for u in wo_units(mc):
                    u()
    return nc


_LEGALIZE_TYPES = None


def _legalize_pe_waits(nc, max_waits=1):
    """walrus' TPB instruction encodings fit very few semaphore waits
    (Matmult: 1; TensorTensor etc. similarly limited) but Tile sometimes
    emits more.  Move the excess onto an InstNoOp inserted just before the
    instruction in the same engine stream — waiting earlier on the same
    engine is always safe."""
    global _LEGALIZE_TYPES
    if _LEGALIZE_TYPES is None:
        _LEGALIZE_TYPES = (
            mybir.InstMatmult, mybir.InstLdweights, mybir.InstTensorTensor,
            mybir.InstTensorCopy, mybir.InstActivation, mybir.InstReciprocal,
            mybir.InstMemset, mybir.InstTensorReduce, mybir.InstIota,
            mybir.InstTensorScalarPtr, mybir.InstISA, mybir.InstDMACopy,
            mybir.InstTensorTensorReduce, mybir.InstDrain,
            mybir.InstDmaTransposeAnt,
        )
    n_fixed = 0
    for fn in nc.m.functions:
        for blk in fn.blocks:
            insts = list(blk.instructions)
            out = []
            for inst in insts:
                si = getattr(inst, "sync_info", None)
                if (
                    isinstance(inst, _LEGALIZE_TYPES)
                    and si is not None
                    and si.on_wait
                    and len(si.on_wait) > max_waits
                ):
                    extra = list(si.on_wait[:-max_waits])
                    keep = list(si.on_wait[-max_waits:])
                    for w in extra:
                        out.append(mybir.InstEventSemaphore(
                            name=nc.get_next_instruction_name(),
                            engine=inst.engine,
                            ins=[],
                            outs=[],
                            sync_info=mybir.SyncInfo(on_wait=[w], on_update=[]),
                            bass_nofuse=True,
                        ))
                    inst.sync_info = mybir.SyncInfo(
                        on_wait=keep, on_update=list(si.on_update)
                    )
                    n_fixed += 1
                out.append(inst)
            blk.instructions = out
    return n_fixed


_NC_CACHE = {}


def _get_nc():
    if "nc" not in _NC_CACHE:
        nc = build_nc()
        _legalize_pe_waits(nc)
        _NC_CACHE["nc"] = nc
    return _NC_CACHE["nc"]


def _make_mask():
    tri = np.triu(np.ones((P, P), np.float32))  # keep j<=c
    return np.ascontiguousarray(
        np.broadcast_to(tri[:, None, :], (P, 2, P))
    ).astype(BF16)


def kernel(x, Wq, Wkv, Wo, **kw):
    x = np.asarray(x, np.float32)
    Wq = np.asarray(Wq, np.float32)
    Wkv = np.asarray(Wkv, np.float32)
    Wo = np.asarray(Wo, np.float32)
    mask = _make_mask()

    in_maps = []
    for c in range(8):
        b = c // 4
        hs = (c % 4) * DQ
        xb = x[b]  # [2048, 1024]
        # xtq[q, p, kc, i] = x[b, q*512+i, kc*128+p]
        xtq = xb.reshape(NIC, IC, KC, P).transpose(0, 3, 2, 1)
        xtq = np.ascontiguousarray(xtq).reshape(NIC, P, KC * IC)
        # wqkv[p, kc, :] = [Wq | Wk | Wv][kc*128+p, shard cols]
        wq_s = Wq[:, hs:hs + DQ].reshape(KC, P, DQ).transpose(1, 0, 2)
        wk_s = Wkv[:, hs:hs + DQ].reshape(KC, P, DQ).transpose(1, 0, 2)
        wv_s = Wkv[:, D + hs:D + hs + DQ].reshape(KC, P, DQ).transpose(1, 0, 2)
        wqkv_h = np.ascontiguousarray(
            np.concatenate([wq_s, wk_s, wv_s], axis=2))
        # wo[p, c2, :] = Wo[hs + c2*128 + p, :]
        wo_s = np.ascontiguousarray(
            Wo[hs:hs + DQ, :].reshape(2, P, D).transpose(1, 0, 2))
        in_maps.append({
            "xtq": xtq.astype(BF16),
            "wqkv": wqkv_h.astype(BF16),
            "wo": wo_s.astype(BF16),
            "mask": mask,
        })

    res = run_bass_kernel_spmd(_get_nc(), in_maps, core_ids=list(range(8)))
    LAST_RESULT["exec_time_ns"] = res.exec_time_ns
    LAST_RESULT["trace"] = res.instructions_and_trace
    LAST_RESULT["profile_json"] = res.profile_json
    parts = [np.asarray(r["out"], np.float32) for r in res.results]
    out = np.stack(
        [parts[0] + parts[1] + parts[2] + parts[3],
         parts[4] + parts[5] + parts[6] + parts[7]], axis=0
    )
    return out
